# revision 4
# baseline (speedup 1.0000x reference)
"""Trainium2 Bass kernel: MeanHinAggregator (GNN message passing).

Reference computation (per batch-head element bh):
    z_r  = mean_n(x_neigh_r[bh, n, :]) @ w_neigh_r          (r = 0, 1)
    out  = relu(concat(x_self[bh] @ w_self, (z0 + z1) / 2) + b)

Strategy (pure data parallel over 8 NeuronCores, batch axis sharded):
  * Per core: B_shard=128, H=10 -> 1280 bh rows.  Row mapping is
    partition-major: bh = 10*p + g (p = SBUF partition, g = group 0..9), so
    per-partition DRAM runs are contiguous ACROSS adjacent groups: x_self
    loads in ONE dma (5 KiB/partition), the output stores in three batched
    dmas, and neighbour groups are loaded in PAIRS with 32 KiB packets
    (vs 16 KiB), which raises per-DMA-engine throughput and halves the
    packet count (less profiler-DMA interference on the straggler engine).
  * The per-core roofline is the 16 DMA engines (~27 GB/s each when busy,
    ~430 GB/s aggregate): 44 MB of fp32 input must stream in.  The two
    neighbour tensors ride separate HWDGE rings (sync = xn0 + x_self,
    scalar = consts + xn1); stores are enqueued after all loads so they
    never head-of-line block the load stream.
  * Mean over 32 neighbours: in-place strided adds on the Vector engine.
    The first add reads fp32 and writes bf16; the rest run in bf16 which
    gets the DVE 2x packed-16-bit mode.  The last group's tiles are loaded
    as two halves with a fold order that lets DVE start before the final
    bytes land, shrinking the pipeline-drain tail.
  * The folded [bh, f] slices are transposed into [f, bh] PSUM layout with
    single-pass bf16 matmuls against a bf16 identity.  Projection matmuls,
    also bf16, use host-precast weights with the 1/(N*NR) mean scale folded
    in.  Bias is added with a K=1 matmul; final ReLU emits fp32.
"""

import numpy as np
import ml_dtypes

import concourse.bacc as bacc
import concourse.bass as bass
import concourse.tile as tile
from concourse import bass_utils, mybir
from concourse._compat import with_exitstack

B, H, N, F = 1024, 10, 32, 128
HALF = 128
D = 2 * HALF
NR = 2
NCORES = 8
BSH = B // NCORES        # 128 batch rows per core
BH = BSH * H             # 1280 bh rows per core
NG = 10                  # groups per core (128 bh rows each)
GF = N * F               # 4096 elements per bh row
F32 = mybir.dt.float32
BF16 = mybir.dt.bfloat16
NPBF16 = ml_dtypes.bfloat16


@with_exitstack
def _tile_kernel(ctx, tc, outs, ins):
    nc = tc.nc
    xn0, xn1, xs, w_s, w0, w1, bvec, ident_d = ins
    (out_d,) = outs

    const = ctx.enter_context(tc.tile_pool(name="const", bufs=1))
    xpool = ctx.enter_context(tc.tile_pool(name="xp", bufs=2))
    fpool = ctx.enter_context(tc.tile_pool(name="fp", bufs=3))
    spool = ctx.enter_context(tc.tile_pool(name="sp", bufs=3))
    ppool = ctx.enter_context(tc.tile_pool(name="ps", bufs=2, space="PSUM"))
    pout = ctx.enter_context(tc.tile_pool(name="po", bufs=2, space="PSUM"))

    # Constants on the scalar ring ahead of the xn1 stream; x_self on the
    # sync ring ahead of xn0 (keeps the two load rings byte-balanced).
    ident = const.tile([128, 128], BF16, tag="ident")
    nc.scalar.dma_start(ident[:], ident_d[:])
    wS_t = const.tile([128, HALF], BF16, tag="wS")
    nc.scalar.dma_start(wS_t[:], w_s[:])
    w0_t = const.tile([128, HALF], BF16, tag="w0")
    nc.scalar.dma_start(w0_t[:], w0[:])
    w1_t = const.tile([128, HALF], BF16, tag="w1")
    nc.scalar.dma_start(w1_t[:], w1[:])
    b_t = const.tile([1, D], BF16, tag="b")
    nc.scalar.dma_start(b_t[:], bvec[:])
    xs_all = const.tile([128, NG * F], F32, tag="xs_all")
    nc.sync.dma_start(xs_all[:], xs[:])
    ones_t = const.tile([1, 128], BF16, tag="ones")
    nc.vector.memset(ones_t[:], 1.0)
    # Cast x_self on the Vector engine (idle until the first neighbour tile
    # lands).  Keeping the ACT engine free of waits here matters: its next
    # instructions enqueue the xn1 pair loads, and the DMA engines serve
    # descriptors in arrival order -- a blocked ACT engine starves the
    # scalar ring and serializes the two load streams.
    xs_bf = const.tile([128, NG * F], BF16, tag="xs_bf")
    nc.vector.tensor_copy(xs_bf[:], xs_all[:])

    obuf = const.tile([128, NG * D], F32, tag="obuf")

    # Neighbour tiles: one [128, 2*4096] tile holds a PAIR of groups
    # (2g, 2g+1); rows 10p+2g and 10p+2g+1 are contiguous in DRAM, so
    # pairs 0..3 load as a single DMA with 32 KiB packets.  The last pair
    # (g8, g9) loads as separate group DMAs -- g9 split in two halves --
    # so the pipeline drains with fine-grained fold overlap.
    def issue_pair(pr):
        t0 = xpool.tile([128, 2 * GF], F32, tag="t0")
        t1 = xpool.tile([128, 2 * GF], F32, tag="t1")
        if pr < 4:
            nc.sync.dma_start(t0[:], xn0[:, 2 * pr * GF:(2 * pr + 2) * GF])
            nc.scalar.dma_start(t1[:], xn1[:, 2 * pr * GF:(2 * pr + 2) * GF])
        else:
            nc.sync.dma_start(t0[:, 0:GF], xn0[:, 8 * GF:9 * GF])
            nc.scalar.dma_start(t1[:, 0:GF], xn1[:, 8 * GF:9 * GF])
            h = GF // 2
            nc.sync.dma_start(t0[:, GF:GF + h], xn0[:, 9 * GF:9 * GF + h])
            nc.scalar.dma_start(t1[:, GF:GF + h], xn1[:, 9 * GF:9 * GF + h])
            nc.sync.dma_start(t0[:, GF + h:2 * GF], xn0[:, 9 * GF + h:10 * GF])
            nc.scalar.dma_start(t1[:, GF + h:2 * GF], xn1[:, 9 * GF + h:10 * GF])
        return t0, t1

    def fold(t, off, tag):
        """Fold the 32 slices of group-tile t[:, off:off+GF] -> bf16 [128, F]."""
        fb = fpool.tile([128, 16 * F], BF16, tag=tag)
        nc.vector.tensor_add(fb[:], t[:, off:off + 16 * F],
                             t[:, off + 16 * F:off + 32 * F])
        for lv in (8, 4, 2, 1):
            nc.vector.tensor_add(fb[:, 0:lv * F], fb[:, 0:lv * F],
                                 fb[:, lv * F:2 * lv * F])
        return fb

    def fold_split(t, off, tag):
        """Same reduction, but half-at-a-time so DVE can start on the first
        half-DMA: each half folds its 16 slices pairwise, then combine."""
        h = GF // 2
        fb = fpool.tile([128, 16 * F], BF16, tag=tag)
        nc.vector.tensor_add(fb[:, 0:8 * F], t[:, off:off + 8 * F],
                             t[:, off + 8 * F:off + 16 * F])
        nc.vector.tensor_add(fb[:, 8 * F:16 * F], t[:, off + h:off + h + 8 * F],
                             t[:, off + h + 8 * F:off + h + 16 * F])
        nc.vector.tensor_add(fb[:, 0:8 * F], fb[:, 0:8 * F], fb[:, 8 * F:16 * F])
        for lv in (4, 2, 1):
            nc.vector.tensor_add(fb[:, 0:lv * F], fb[:, 0:lv * F],
                                 fb[:, lv * F:2 * lv * F])
        return fb

    def compute_group(g, fb0, fb1):
        pacc = ppool.tile([128, 3 * 128], F32, tag="pacc")
        nc.tensor.matmul(pacc[:, 0:128], fb0[:, 0:F], ident[:],
                         start=True, stop=True)
        nc.tensor.matmul(pacc[:, 128:256], fb1[:, 0:F], ident[:],
                         start=True, stop=True)
        nc.tensor.matmul(pacc[:, 256:384], xs_bf[:, g * F:(g + 1) * F],
                         ident[:], start=True, stop=True)

        sacc = spool.tile([128, 3 * 128], BF16, tag="sacc")
        nc.scalar.activation(sacc[:], pacc[:], mybir.ActivationFunctionType.Copy)

        po = pout.tile([128, D], F32, tag="po")
        nc.tensor.matmul(po[:, 0:HALF], sacc[:, 256:384], wS_t[:],
                         start=True, stop=False)
        nc.tensor.matmul(po[:, 0:HALF], ones_t[:], b_t[:, 0:HALF],
                         start=False, stop=True)
        nc.tensor.matmul(po[:, HALF:D], sacc[:, 0:128], w0_t[:],
                         start=True, stop=False)
        nc.tensor.matmul(po[:, HALF:D], sacc[:, 128:256], w1_t[:],
                         start=False, stop=False)
        nc.tensor.matmul(po[:, HALF:D], ones_t[:], b_t[:, HALF:D],
                         start=False, stop=True)

        nc.scalar.activation(obuf[:, g * D:(g + 1) * D], po[:],
                             mybir.ActivationFunctionType.Relu)

    pending = [issue_pair(0), issue_pair(1)]
    for pr in range(5):
        t0, t1 = pending.pop(0)
        if pr + 2 < 5:
            pending.append(issue_pair(pr + 2))
        if pr < 4:
            for k in (0, 1):
                g = 2 * pr + k
                fb0 = fold(t0, k * GF, "fb0")
                fb1 = fold(t1, k * GF, "fb1")
                compute_group(g, fb0, fb1)
        else:
            fb0 = fold(t0, 0, "fb0")
            fb1 = fold(t1, 0, "fb1")
            compute_group(8, fb0, fb1)
            # g9: halves fold as they land; t1 first (scalar ring drains
            # marginally earlier), then t0.
            fb1 = fold_split(t1, GF, "fb1")
            fb0 = fold_split(t0, GF, "fb0")
            compute_group(9, fb0, fb1)

    # Stores: enqueued after every load so they never block the stream.
    # relu(g4)/relu(g8) gate them via semaphores, not ring order.
    nc.sync.dma_start(out_d[:, 0:5 * D], obuf[:, 0:5 * D])
    nc.sync.dma_start(out_d[:, 5 * D:9 * D], obuf[:, 5 * D:9 * D])
    nc.sync.dma_start(out_d[:, 9 * D:NG * D], obuf[:, 9 * D:NG * D])


def build_nc():
    nc = bacc.Bacc("TRN2", target_bir_lowering=False, debug=False)
    # bh rows are partition-major: dram row index = 10*p + g.
    xn0 = nc.dram_tensor("xn0", [128, NG * GF], F32, kind="ExternalInput")
    xn1 = nc.dram_tensor("xn1", [128, NG * GF], F32, kind="ExternalInput")
    xs = nc.dram_tensor("xs", [128, NG * F], F32, kind="ExternalInput")
    w_s = nc.dram_tensor("w_s", [F, HALF], BF16, kind="ExternalInput")
    w0 = nc.dram_tensor("w0", [F, HALF], BF16, kind="ExternalInput")
    w1 = nc.dram_tensor("w1", [F, HALF], BF16, kind="ExternalInput")
    bvec = nc.dram_tensor("bvec", [1, D], BF16, kind="ExternalInput")
    ident_d = nc.dram_tensor("ident", [128, 128], BF16, kind="ExternalInput")
    out = nc.dram_tensor("out", [128, NG * D], F32, kind="ExternalOutput")

    ins = [t.ap() for t in (xn0, xn1, xs, w_s, w0, w1, bvec, ident_d)]
    with tile.TileContext(nc) as tc:
        _tile_kernel(tc, [out.ap()], ins)
    nc.compile()
    return nc


def make_in_maps(x_self, x_neigh_0, x_neigh_1, w_self, w_neigh_0, w_neigh_1, b):
    """Shard full inputs into per-core input maps (batch axis, 8 ways)."""
    x_self = np.ascontiguousarray(np.asarray(x_self, dtype=np.float32))
    x_neigh_0 = np.ascontiguousarray(np.asarray(x_neigh_0, dtype=np.float32))
    x_neigh_1 = np.ascontiguousarray(np.asarray(x_neigh_1, dtype=np.float32))
    scale = np.float32(1.0 / (N * NR))
    w_s = np.asarray(w_self, dtype=np.float32).astype(NPBF16)
    w0 = (np.asarray(w_neigh_0, dtype=np.float32) * scale).astype(NPBF16)
    w1 = (np.asarray(w_neigh_1, dtype=np.float32) * scale).astype(NPBF16)
    bvec = np.asarray(b, dtype=np.float32).reshape(1, D).astype(NPBF16)
    ident = np.eye(128, dtype=np.float32).astype(NPBF16)

    in_maps = []
    for c in range(NCORES):
        bs = slice(c * BSH, (c + 1) * BSH)
        in_maps.append({
            "xn0": np.ascontiguousarray(x_neigh_0[bs].reshape(128, NG * GF)),
            "xn1": np.ascontiguousarray(x_neigh_1[bs].reshape(128, NG * GF)),
            "xs": np.ascontiguousarray(x_self[bs].reshape(128, NG * F)),
            "w_s": w_s, "w0": w0, "w1": w1, "bvec": bvec, "ident": ident,
        })
    return in_maps


_NC_CACHE = None


def kernel(x_self, x_neigh_0, x_neigh_1, w_self, w_neigh_0, w_neigh_1, b):
    global _NC_CACHE
    if _NC_CACHE is None:
        _NC_CACHE = build_nc()
    in_maps = make_in_maps(x_self, x_neigh_0, x_neigh_1,
                           w_self, w_neigh_0, w_neigh_1, b)
    res = bass_utils.run_bass_kernel_spmd(
        _NC_CACHE, in_maps, core_ids=list(range(NCORES)))
    out = np.concatenate([r["out"].reshape(BH, D) for r in res.results], axis=0)
    return out.reshape(B, H, D)
